# revision 1
# baseline (speedup 1.0000x reference)
"""Causal self-attention (B=4, T=2048, C=1024, H=16) on 8 TRN2 NeuronCores.

Sharding: core = (batch b, head-group g) with b = core//2, g = core%2.
Each core computes, for its batch and its 8 heads:
  QKV projection (W_qkv column shard), causal attention, and a PARTIAL
  output projection (W_pr row shard).  Host sums the two partials per
  batch and adds b_pr.

On-chip layout (per core):
  xT   [C, T]   : x[b].T               (DMA'd per 512-wide t-chunk)
  Q^T/K^T [512, T] : computed transposed (lhsT = W slice, rhs = xT)
  V    [T, 512] : computed natural     (lhsT = xT slice,  rhs = Wv)
  Attention per chunk j (q in [512j, 512j+512)):
    S^T blocks [k-tile 128, q 512] = K_h^T.T-free matmul (contraction d=64)
    exp on ACT (scale=1/8) -> A^T bf16, causal mask via gpsimd.affine_select
    Y^T[65, q] += [V_h | ones].T @ A^T   (row 64 = softmax denominators)
    normalize with reciprocal_approx_accurate + gpsimd.partition_broadcast
  Projection: out^T[c_out, t] += W_pr_shard.T @ Y^T  (fp32r)
"""

import numpy as np

import concourse.bass as bass
import concourse.mybir as mybir
import concourse.tile as tile
from concourse.bass_utils import run_bass_kernel_spmd


def _split_multiwaits(nc: bass.Bass, max_waits: int = 1) -> None:
    """The walrus build in this container rejects >max_waits sync-waits on an
    instruction ("Too many sync wait commands").  Move extra waits onto
    same-engine NoOps inserted immediately before the instruction — the
    engine blocks on each NoOp's wait first, so semantics are unchanged."""
    n = 0
    for fn in nc.m.functions:
        for blk in fn.blocks:
            out = []
            for inst in blk.instructions:
                si = getattr(inst, "sync_info", None)
                waits = list(si.on_wait) if si is not None and si.on_wait else []
                if len(waits) > max_waits:
                    keep = waits[-max_waits:]
                    for w in waits[: -max_waits]:
                        nop = mybir.InstNoOp(name=f"{inst.name}-w{n}", ins=[], outs=[])
                        n += 1
                        nop.engine = inst.engine
                        nop.sync_info = mybir.SyncInfo(on_wait=[w], on_update=[])
                        out.append(nop)
                    inst.sync_info = mybir.SyncInfo(
                        on_wait=keep, on_update=list(si.on_update or [])
                    )
                out.append(inst)
            blk.instructions = out

AF = mybir.ActivationFunctionType
ALU = mybir.AluOpType

F32 = mybir.dt.float32
F32R = mybir.dt.float32r
BF16 = mybir.dt.bfloat16

B, T_FULL, C = 4, 2048, 1024
H, HD = 16, 64
HPC = 8              # heads per core
GC = HPC * HD        # 512: per-core head-group width
P = 128
CH = 512             # q-chunk width
NKC = C // P         # 8 k-tiles over the C contraction

NP_BF16 = mybir.dt.np(BF16)


def build_attention(T: int = T_FULL, split_waits: bool = True) -> bass.Bass:
    assert T % CH == 0
    nch = T // CH        # q-chunks
    ntt = T // P         # t-tiles

    nc = bass.Bass("TRN2", debug=False, num_devices=8)

    xT_d = nc.dram_tensor("xT", [C, T], BF16, kind="ExternalInput").ap()
    wq_d = nc.dram_tensor("wq", [C, GC], BF16, kind="ExternalInput").ap()
    wk_d = nc.dram_tensor("wk", [C, GC], BF16, kind="ExternalInput").ap()
    wv_d = nc.dram_tensor("wv", [C, GC], BF16, kind="ExternalInput").ap()
    bq_d = nc.dram_tensor("bq", [GC], F32, kind="ExternalInput").ap()
    bk_d = nc.dram_tensor("bk", [GC], F32, kind="ExternalInput").ap()
    bv_d = nc.dram_tensor("bv", [GC], BF16, kind="ExternalInput").ap()
    wpr_d = nc.dram_tensor("wpr", [GC, C], BF16, kind="ExternalInput").ap()
    out_d = nc.dram_tensor("outT", [C, T], F32, kind="ExternalOutput").ap()

    with tile.TileContext(nc) as tc:
        with (
            tc.tile_pool(name="singles", bufs=1) as singles,
            tc.tile_pool(name="xt", bufs=2) as xt_pool,
            tc.tile_pool(name="qt", bufs=3) as qt_pool,
            tc.tile_pool(name="at", bufs=5) as at_pool,
            tc.tile_pool(name="yt", bufs=3) as yt_pool,
            tc.tile_pool(name="dd", bufs=4) as d_pool,
            tc.tile_pool(name="bc", bufs=4) as bc_pool,
            tc.tile_pool(name="ot", bufs=3) as out_pool,
            tc.tile_pool(name="swps", bufs=2, space="PSUM") as sweep_ps,
            tc.tile_pool(name="scps", bufs=2, space="PSUM") as sc_ps,
            tc.tile_pool(name="yps", bufs=2, space="PSUM") as y_ps,
        ):
            # ---- resident tensors ----
            wq_sb = singles.tile([P, NKC, GC], BF16)
            wk_sb = singles.tile([P, NKC, GC], BF16)
            wv_sb = singles.tile([P, NKC, GC], BF16)
            nc.sync.dma_start(out=wq_sb, in_=wq_d.rearrange("(kc p) n -> p kc n", p=P))
            nc.sync.dma_start(out=wk_sb, in_=wk_d.rearrange("(kc p) n -> p kc n", p=P))
            nc.sync.dma_start(out=wv_sb, in_=wv_d.rearrange("(kc p) n -> p kc n", p=P))
            wpr_sb = singles.tile([P, GC // P, C], BF16)
            nc.sync.dma_start(
                out=wpr_sb, in_=wpr_d.rearrange("(kp p) m -> p kp m", p=P)
            )
            bqk_sb = singles.tile([P, 2, GC // P], F32)
            nc.sync.dma_start(
                out=bqk_sb[:, 0, :], in_=bq_d.rearrange("(m p) -> p m", p=P)
            )
            nc.sync.dma_start(
                out=bqk_sb[:, 1, :], in_=bk_d.rearrange("(m p) -> p m", p=P)
            )
            bv_sb = singles.tile([1, GC], BF16)
            nc.sync.dma_start(out=bv_sb, in_=bv_d.rearrange("(o n) -> o n", o=1))
            ones_sb = singles.tile([1, P], BF16)
            nc.vector.memset(ones_sb, 1.0)
            ones64_sb = singles.tile([P, HD], BF16)
            nc.vector.memset(ones64_sb, 1.0)

            kt_sb = singles.tile([P, GC // P, T], BF16)       # K^T, filled per chunk
            v_sb = singles.tile([P, ntt, HPC, HD + 1], BF16)  # [V | 1] per t-tile/head
            nc.vector.memset(v_sb[:, :, :, HD : HD + 1], 1.0)

            for j in range(nch):
                nkt = 4 * (j + 1)           # k-tiles valid for this q-chunk
                tsl = slice(j * CH, (j + 1) * CH)

                # ---- sweep: QT/KT chunk-j columns, V t-tiles 4j..4j+3 ----
                xt_t = xt_pool.tile([P, NKC, CH], BF16)
                nc.sync.dma_start(
                    out=xt_t, in_=xT_d[:, tsl].rearrange("(kc p) t -> p kc t", p=P)
                )

                qt_t = qt_pool.tile([P, GC // P, CH], BF16)
                for mq in range(GC // P):
                    ps = sweep_ps.tile([P, CH], F32)
                    for kc in range(NKC):
                        nc.tensor.matmul(
                            ps,
                            lhsT=wq_sb[:, kc, mq * P : (mq + 1) * P],
                            rhs=xt_t[:, kc, :],
                            start=(kc == 0),
                            stop=(kc == NKC - 1),
                        )
                    nc.vector.tensor_scalar_add(
                        out=qt_t[:, mq, :], in0=ps, scalar1=bqk_sb[:, 0, mq : mq + 1]
                    )
                for mk in range(GC // P):
                    ps = sweep_ps.tile([P, CH], F32)
                    for kc in range(NKC):
                        nc.tensor.matmul(
                            ps,
                            lhsT=wk_sb[:, kc, mk * P : (mk + 1) * P],
                            rhs=xt_t[:, kc, :],
                            start=(kc == 0),
                            stop=(kc == NKC - 1),
                        )
                    nc.vector.tensor_scalar_add(
                        out=kt_sb[:, mk, tsl], in0=ps, scalar1=bqk_sb[:, 1, mk : mk + 1]
                    )
                for tl in range(4):
                    tt = 4 * j + tl
                    ps = sweep_ps.tile([P, GC], F32)
                    for kc in range(NKC):
                        nc.tensor.matmul(
                            ps,
                            lhsT=xt_t[:, kc, tl * P : (tl + 1) * P],
                            rhs=wv_sb[:, kc, :],
                            start=(kc == 0),
                            stop=False,
                        )
                    # bias row: V += ones.T @ bv
                    nc.tensor.matmul(
                        ps, lhsT=ones_sb, rhs=bv_sb, start=False, stop=True
                    )
                    nc.vector.tensor_copy(
                        v_sb[:, tt, :, 0:HD], ps.rearrange("p (h d) -> p h d", h=HPC)
                    )

                # ---- attention for chunk j: head pairs interleaved so the
                # two K=64 scores matmuls land in different PE row groups
                # (base partitions 0/64) and run concurrently, with the next
                # LDWEIGHTS pulled ahead by the PE reorder window. ----
                yt_t = yt_pool.tile([P, GC // P, CH], BF16)
                for hp in range(HPC // 2):
                    mk = hp
                    yps0 = y_ps.tile([P, CH], F32, tag="yps")  # rows 0..64
                    yps1 = y_ps.tile([P, CH], F32, tag="yps")
                    ypss = [yps0, yps1]
                    for ki in range(nkt):
                        sc = sc_ps.tile([P, 2 * CH], F32)
                        for u in range(2):  # u = head parity; po = 64*u
                            po = HD * u
                            nc.tensor.matmul(
                                sc[:, u * CH : (u + 1) * CH],
                                lhsT=kt_sb[po : po + HD, mk, ki * P : (ki + 1) * P],
                                rhs=qt_t[po : po + HD, mk, :],
                                start=True,
                                stop=True,
                            )
                        at_t = at_pool.tile([P, 2 * CH], BF16)
                        nc.scalar.activation(at_t, sc, AF.Exp, scale=0.125)
                        r = ki - 4 * j
                        if r >= 0:  # diagonal block: zero where q < k
                            for u in range(2):
                                blk = at_t[:, u * CH : (u + 1) * CH]
                                nc.gpsimd.affine_select(
                                    out=blk,
                                    in_=blk,
                                    pattern=[[1, CH]],
                                    compare_op=ALU.is_ge,
                                    fill=0.0,
                                    base=-P * r,
                                    channel_multiplier=-1,
                                )
                        for u in range(2):
                            nc.tensor.matmul(
                                ypss[u][0 : HD + 1, :],
                                lhsT=v_sb[:, ki, 2 * hp + u, :],
                                rhs=at_t[:, u * CH : (u + 1) * CH],
                                start=(ki == 0),
                                stop=(ki == nkt - 1),
                            )
                    for u in range(2):
                        po = HD * u
                        yps = ypss[u]
                        # softmax denominator: Dinv = exp(-ln D) on ACT
                        # (custom-DVE reciprocal doesn't encode under this
                        # walrus build), broadcast across partitions via a
                        # rank-1 bf16 matmul, then one fused
                        # PSUM-read * broadcast -> bf16 Y^T write on DVE.
                        dr_t = d_pool.tile([P, CH], F32)
                        dr2_t = d_pool.tile([P, CH], BF16, tag="dr2")
                        nc.scalar.activation(
                            dr_t[HD : HD + 1, :], yps[HD : HD + 1, :], AF.Ln
                        )
                        nc.scalar.activation(
                            dr2_t[HD : HD + 1, :],
                            dr_t[HD : HD + 1, :],
                            AF.Exp,
                            scale=-1.0,
                        )
                        bc_ps = sweep_ps.tile([HD, CH], F32, tag="ps")
                        nc.tensor.matmul(
                            bc_ps,
                            lhsT=ones64_sb[HD : HD + 1, :],
                            rhs=dr2_t[HD : HD + 1, :],
                            start=True,
                            stop=True,
                        )
                        bc_t = bc_pool.tile([HD, CH], F32)
                        nc.vector.tensor_copy(bc_t, bc_ps)
                        nc.vector.tensor_mul(
                            yt_t[po : po + HD, mk, :], yps[0:HD, :], bc_t
                        )

                # ---- partial output projection for chunk j (bf16) ----
                for m in range(C // P):
                    pp = sweep_ps.tile([P, CH], F32, tag="ps")
                    for kp in range(GC // P):
                        nc.tensor.matmul(
                            pp,
                            lhsT=wpr_sb[:, kp, m * P : (m + 1) * P],
                            rhs=yt_t[:, kp, :],
                            start=(kp == 0),
                            stop=(kp == GC // P - 1),
                        )
                    ot = out_pool.tile([P, CH], F32)
                    nc.vector.tensor_copy(ot, pp)
                    nc.sync.dma_start(out=out_d[m * P : (m + 1) * P, tsl], in_=ot)

    if split_waits:  # breaks CoreSim's sem bookkeeping; needed for walrus
        _split_multiwaits(nc)
    return nc


def make_in_maps(x, W_qkv, b_qkv, W_pr):
    """Shard FULL inputs into the 8 per-core input dicts."""
    x = np.asarray(x, dtype=np.float32)
    W_qkv = np.asarray(W_qkv, dtype=np.float32)
    b_qkv = np.asarray(b_qkv, dtype=np.float32)
    W_pr = np.asarray(W_pr, dtype=np.float32)
    in_maps = []
    for core in range(8):
        b, g = divmod(core, 2)
        sl = slice(g * GC, (g + 1) * GC)
        in_maps.append(
            {
                "xT": np.ascontiguousarray(x[b].T).astype(NP_BF16),
                "wq": np.ascontiguousarray(W_qkv[:, 0 * C :][:, sl]).astype(NP_BF16),
                "wk": np.ascontiguousarray(W_qkv[:, 1 * C :][:, sl]).astype(NP_BF16),
                "wv": np.ascontiguousarray(W_qkv[:, 2 * C :][:, sl]).astype(NP_BF16),
                "bq": np.ascontiguousarray(b_qkv[0 * C :][sl]),
                "bk": np.ascontiguousarray(b_qkv[1 * C :][sl]),
                "bv": np.ascontiguousarray(b_qkv[2 * C :][sl]).astype(NP_BF16),
                "wpr": np.ascontiguousarray(W_pr[sl, :]).astype(NP_BF16),
            }
        )
    return in_maps


def assemble_output(parts, b_pr):
    """parts: 8 per-core outT [C, T] partials -> full [B, T, C] output."""
    b_pr = np.asarray(b_pr, dtype=np.float32)
    out = np.empty((B, T_FULL, C), dtype=np.float32)
    for b in range(B):
        out[b] = (parts[2 * b] + parts[2 * b + 1]).T + b_pr
    return out


_CACHE = {}


def kernel(x, W_qkv, b_qkv, W_pr, b_pr):
    if "nc" not in _CACHE:
        _CACHE["nc"] = build_attention(T_FULL)
    in_maps = make_in_maps(x, W_qkv, b_qkv, W_pr)
    res = run_bass_kernel_spmd(_CACHE["nc"], in_maps, core_ids=list(range(8)))
    parts = [r["outT"] for r in res.results]
    return assemble_output(parts, b_pr)



# revision 4
# speedup vs baseline: 1.1429x; 1.1429x over previous
"""Causal self-attention (B=4, T=2048, C=1024, H=16) on 8 TRN2 NeuronCores.

Sharding: core = (batch b, head-group g) with b = core//2, g = core%2.
Each core computes, for its batch and its 8 heads:
  QKV projection (W_qkv column shard), causal attention, and a PARTIAL
  output projection (W_pr row shard).  Host sums the two partials per
  batch and adds b_pr.

On-chip layout (per core):
  xT   [C, T]   : x[b].T               (DMA'd per 512-wide t-chunk)
  Q^T/K^T [512, T] : computed transposed (lhsT = W slice, rhs = xT)
  V    [T, 512] : computed natural     (lhsT = xT slice,  rhs = Wv)
  Attention per chunk j (q in [512j, 512j+512)):
    S^T blocks [k-tile 128, q 512] = K_h^T matmul pair packed in PE row
    groups (contraction d=64 at partition bases 0/64), exp on ACT
    (scale=1/8) -> A^T bf16, causal mask via gpsimd.affine_select over
    only the columns that can be masked.
    Y^T[65, q] += [V_h | ones].T @ A^T   (row 64 = softmax denominators)
  Per head pair: denominators to partitions {0, 64} of one staging
  tile, ONE Ln + ONE Exp(-x) (ACT cost is per-column, so batching
  partitions is free), 1/D broadcast via a packed pair of rank-1
  matmuls, then two PSUM*PSUM -> bf16 Y^T writes on DVE.
  Projection: out^T[c_out, t] += W_pr_shard.T @ Y^T

Software pipeline: the QKV sweep for chunk j+1 and the output
projection for chunk j-1 are interleaved into the attention
instruction stream of chunk j, so the PE's exp/denominator wait gaps
are filled with independent matmul work.
"""

import numpy as np

import concourse.bass as bass
import concourse.mybir as mybir
import concourse.tile as tile
from concourse.bass_utils import run_bass_kernel_spmd


def _split_multiwaits(nc: bass.Bass, max_waits: int = 1) -> None:
    """The walrus build in this container rejects >max_waits sync-waits on an
    instruction ("Too many sync wait commands").  Move extra waits onto
    same-engine NoOps inserted immediately before the instruction — the
    engine blocks on each NoOp's wait first, so semantics are unchanged."""
    n = 0
    for fn in nc.m.functions:
        for blk in fn.blocks:
            out = []
            for inst in blk.instructions:
                si = getattr(inst, "sync_info", None)
                waits = list(si.on_wait) if si is not None and si.on_wait else []
                if len(waits) > max_waits:
                    keep = waits[-max_waits:]
                    for w in waits[: -max_waits]:
                        nop = mybir.InstNoOp(name=f"{inst.name}-w{n}", ins=[], outs=[])
                        n += 1
                        nop.engine = inst.engine
                        nop.sync_info = mybir.SyncInfo(on_wait=[w], on_update=[])
                        out.append(nop)
                    inst.sync_info = mybir.SyncInfo(
                        on_wait=keep, on_update=list(si.on_update or [])
                    )
                out.append(inst)
            blk.instructions = out

AF = mybir.ActivationFunctionType
ALU = mybir.AluOpType

F32 = mybir.dt.float32
F32R = mybir.dt.float32r
BF16 = mybir.dt.bfloat16

B, T_FULL, C = 4, 2048, 1024
H, HD = 16, 64
HPC = 8              # heads per core
GC = HPC * HD        # 512: per-core head-group width
P = 128
CH = 512             # q-chunk width
NKC = C // P         # 8 k-tiles over the C contraction

NP_BF16 = mybir.dt.np(BF16)


def build_attention(T: int = T_FULL, split_waits: bool = True) -> bass.Bass:
    assert T % CH == 0
    nch = T // CH        # q-chunks
    ntt = T // P         # t-tiles

    nc = bass.Bass("TRN2", debug=False, num_devices=8)

    xT_d = nc.dram_tensor("xT", [C, T], BF16, kind="ExternalInput").ap()
    wq_d = nc.dram_tensor("wq", [C, GC], BF16, kind="ExternalInput").ap()
    wk_d = nc.dram_tensor("wk", [C, GC], BF16, kind="ExternalInput").ap()
    wv_d = nc.dram_tensor("wv", [C, GC], BF16, kind="ExternalInput").ap()
    bq_d = nc.dram_tensor("bq", [GC], F32, kind="ExternalInput").ap()
    bk_d = nc.dram_tensor("bk", [GC], F32, kind="ExternalInput").ap()
    bv_d = nc.dram_tensor("bv", [GC], BF16, kind="ExternalInput").ap()
    wpr_d = nc.dram_tensor("wpr", [GC, C], BF16, kind="ExternalInput").ap()
    out_d = nc.dram_tensor("outT", [C, T], F32, kind="ExternalOutput").ap()

    with tile.TileContext(nc) as tc:
        with (
            tc.tile_pool(name="singles", bufs=1) as singles,
            tc.tile_pool(name="xt", bufs=2) as xt_pool,
            tc.tile_pool(name="qt", bufs=2) as qt_pool,
            tc.tile_pool(name="at", bufs=5) as at_pool,
            tc.tile_pool(name="yt", bufs=2) as yt_pool,
            tc.tile_pool(name="dd", bufs=3) as d_pool,
            tc.tile_pool(name="ot", bufs=3) as out_pool,
            tc.tile_pool(name="swps", bufs=2, space="PSUM") as sweep_ps,
            tc.tile_pool(name="scps", bufs=2, space="PSUM") as sc_ps,
            tc.tile_pool(name="yps", bufs=2, space="PSUM") as y_ps,
        ):
            # ---- resident tensors; DMA issue order = priority order ----
            wq_sb = singles.tile([P, NKC, GC], BF16)
            nc.sync.dma_start(out=wq_sb, in_=wq_d.rearrange("(kc p) n -> p kc n", p=P))
            bqk_sb = singles.tile([P, 2, GC // P], F32)
            nc.sync.dma_start(
                out=bqk_sb[:, 0, :], in_=bq_d.rearrange("(m p) -> p m", p=P)
            )
            nc.sync.dma_start(
                out=bqk_sb[:, 1, :], in_=bk_d.rearrange("(m p) -> p m", p=P)
            )
            wk_sb = singles.tile([P, NKC, GC], BF16)
            wv_sb = singles.tile([P, NKC, GC], BF16)

            bv_sb = singles.tile([1, GC], BF16)
            wpr_sb = singles.tile([P, GC // P, C], BF16)

            ones_sb = singles.tile([1, P], BF16)
            nc.vector.memset(ones_sb, 1.0)
            ones64_sb = singles.tile([P, HD], BF16)
            nc.vector.memset(ones64_sb, 1.0)

            kt_sb = singles.tile([P, GC // P, T], BF16)       # K^T, filled per chunk
            v_sb = singles.tile([P, ntt, HPC, HD + 1], BF16)  # [V | 1] per t-tile/head
            nc.vector.memset(v_sb[:, :, :, HD : HD + 1], 1.0)

            chunk_qt = {}   # j -> qt tile
            chunk_yt = {}   # j -> yt tile

            def sweep_stream(j, first=False):
                """QT/KT chunk-j columns, V t-tiles 4j..4j+3.  Yields at
                unit boundaries (one PSUM pass-group per unit)."""
                tsl = slice(j * CH, (j + 1) * CH)
                xt_t = xt_pool.tile([P, NKC, CH], BF16)
                nc.sync.dma_start(
                    out=xt_t, in_=xT_d[:, tsl].rearrange("(kc p) t -> p kc t", p=P)
                )
                if first:
                    # remaining residents, behind xt0 on the sync queue
                    nc.sync.dma_start(
                        out=wk_sb, in_=wk_d.rearrange("(kc p) n -> p kc n", p=P)
                    )
                    nc.sync.dma_start(
                        out=wv_sb, in_=wv_d.rearrange("(kc p) n -> p kc n", p=P)
                    )
                    nc.sync.dma_start(
                        out=bv_sb, in_=bv_d.rearrange("(o n) -> o n", o=1)
                    )
                    nc.sync.dma_start(
                        out=wpr_sb, in_=wpr_d.rearrange("(kp p) m -> p kp m", p=P)
                    )
                yield

                qt_t = qt_pool.tile([P, GC // P, CH], BF16)
                chunk_qt[j] = qt_t
                for mq in range(GC // P):
                    ps = sweep_ps.tile([P, CH], F32, tag="ps")
                    for kc in range(NKC):
                        nc.tensor.matmul(
                            ps,
                            lhsT=wq_sb[:, kc, mq * P : (mq + 1) * P],
                            rhs=xt_t[:, kc, :],
                            start=(kc == 0),
                            stop=(kc == NKC - 1),
                        )
                    nc.vector.tensor_scalar_add(
                        out=qt_t[:, mq, :], in0=ps, scalar1=bqk_sb[:, 0, mq : mq + 1]
                    )
                    yield
                for mk in range(GC // P):
                    ps = sweep_ps.tile([P, CH], F32, tag="ps")
                    for kc in range(NKC):
                        nc.tensor.matmul(
                            ps,
                            lhsT=wk_sb[:, kc, mk * P : (mk + 1) * P],
                            rhs=xt_t[:, kc, :],
                            start=(kc == 0),
                            stop=(kc == NKC - 1),
                        )
                    nc.vector.tensor_scalar_add(
                        out=kt_sb[:, mk, tsl], in0=ps, scalar1=bqk_sb[:, 1, mk : mk + 1]
                    )
                    yield
                for tl in range(4):
                    tt = 4 * j + tl
                    ps = sweep_ps.tile([P, GC], F32, tag="ps")
                    for kc in range(NKC):
                        nc.tensor.matmul(
                            ps,
                            lhsT=xt_t[:, kc, tl * P : (tl + 1) * P],
                            rhs=wv_sb[:, kc, :],
                            start=(kc == 0),
                            stop=False,
                        )
                    # bias row: V += ones.T @ bv
                    nc.tensor.matmul(
                        ps, lhsT=ones_sb, rhs=bv_sb, start=False, stop=True
                    )
                    nc.vector.tensor_copy(
                        v_sb[:, tt, :, 0:HD], ps.rearrange("p (h d) -> p h d", h=HPC)
                    )
                    yield

            def att_stream(j):
                """Attention for chunk j.  Head pairs at PE row groups 0/64."""
                nkt = 4 * (j + 1)
                yt_t = yt_pool.tile([P, GC // P, CH], BF16)
                chunk_yt[j] = yt_t
                qt_t = chunk_qt[j]
                for hp in range(HPC // 2):
                    mk = hp
                    yps0 = y_ps.tile([P, CH], F32, tag="yps")  # rows 0..64
                    yps1 = y_ps.tile([P, CH], F32, tag="yps")
                    ypss = [yps0, yps1]
                    for ki in range(nkt):
                        sc = sc_ps.tile([P, 2 * CH], F32)
                        for u in range(2):  # u = head parity; po = 64*u
                            po = HD * u
                            nc.tensor.matmul(
                                sc[:, u * CH : (u + 1) * CH],
                                lhsT=kt_sb[po : po + HD, mk, ki * P : (ki + 1) * P],
                                rhs=qt_t[po : po + HD, mk, :],
                                start=True,
                                stop=True,
                            )
                        at_t = at_pool.tile([P, 2 * CH], BF16)
                        nc.scalar.activation(at_t, sc, AF.Exp, scale=0.125)
                        r = ki - 4 * j
                        if r >= 0:  # diagonal block: zero where q < k.
                            # only columns q < 128(r+1) can be masked.
                            w = P * (r + 1)
                            for u in range(2):
                                blk = at_t[:, u * CH : u * CH + w]
                                nc.gpsimd.affine_select(
                                    out=blk,
                                    in_=blk,
                                    pattern=[[1, w]],
                                    compare_op=ALU.is_ge,
                                    fill=0.0,
                                    base=-P * r,
                                    channel_multiplier=-1,
                                )
                        for u in range(2):
                            nc.tensor.matmul(
                                ypss[u][0 : HD + 1, :],
                                lhsT=v_sb[:, ki, 2 * hp + u, :],
                                rhs=at_t[:, u * CH : (u + 1) * CH],
                                start=(ki == 0),
                                stop=(ki == nkt - 1),
                            )
                        yield
                    # ---- head-pair softmax denominators, batched ----
                    # D rows to partitions {0, 64} of one staging tile; ACT
                    # ops cost per-column, so one Ln + one Exp covers both.
                    dsb = d_pool.tile([P, CH], F32, tag="dsb")
                    nc.vector.tensor_copy(dsb[0:1, :], yps0[HD : HD + 1, :])
                    nc.vector.tensor_copy(dsb[64:65, :], yps1[HD : HD + 1, :])
                    dln = d_pool.tile([P, CH], F32, tag="dln")
                    nc.scalar.activation(dln[0:65, :], dsb[0:65, :], AF.Ln)
                    dinv = d_pool.tile([P, CH], BF16, tag="dinv")
                    nc.scalar.activation(
                        dinv[0:65, :], dln[0:65, :], AF.Exp, scale=-1.0
                    )
                    # 1/D broadcast: packed pair of rank-1 matmuls
                    bc_ps = sweep_ps.tile([P, CH], F32, tag="ps")
                    nc.tensor.matmul(
                        bc_ps[0:HD, :],
                        lhsT=ones64_sb[0:1, :],
                        rhs=dinv[0:1, :],
                        start=True,
                        stop=True,
                    )
                    nc.tensor.matmul(
                        bc_ps[HD : 2 * HD, :],
                        lhsT=ones64_sb[64:65, :],
                        rhs=dinv[64:65, :],
                        start=True,
                        stop=True,
                    )
                    # walrus rejects tensor_tensor with two PSUM sources, so
                    # stage the broadcast through SBUF (one copy per pair)
                    bc_t = d_pool.tile([P, CH], F32, tag="bc")
                    nc.vector.tensor_copy(bc_t[0 : 2 * HD, :], bc_ps[0 : 2 * HD, :])
                    nc.vector.tensor_mul(
                        yt_t[0:HD, mk, :], yps0[0:HD, :], bc_t[0:HD, :]
                    )
                    nc.vector.tensor_mul(
                        yt_t[HD:P, mk, :], yps1[0:HD, :], bc_t[HD : 2 * HD, :]
                    )
                    yield

            def proj_stream(j):
                """Partial output projection for chunk j (bf16)."""
                tsl = slice(j * CH, (j + 1) * CH)
                yt_t = chunk_yt[j]
                for m in range(C // P):
                    pp = sweep_ps.tile([P, CH], F32, tag="ps")
                    for kp in range(GC // P):
                        nc.tensor.matmul(
                            pp,
                            lhsT=wpr_sb[:, kp, m * P : (m + 1) * P],
                            rhs=yt_t[:, kp, :],
                            start=(kp == 0),
                            stop=(kp == GC // P - 1),
                        )
                    ot = out_pool.tile([P, CH], F32)
                    nc.vector.tensor_copy(ot, pp)
                    nc.sync.dma_start(out=out_d[m * P : (m + 1) * P, tsl], in_=ot)
                    yield

            def drain(it):
                for _ in it:
                    pass

            # ---- schedule ----
            drain(sweep_stream(0, first=True))
            for j in range(nch):
                fills = []
                if j + 1 < nch:
                    fills.append(sweep_stream(j + 1))
                if j >= 1:
                    fills.append(proj_stream(j - 1))
                # merged fill iterator, round-robin across fill streams
                def merged(streams):
                    while streams:
                        nxt = []
                        for s in streams:
                            try:
                                next(s)
                            except StopIteration:
                                continue
                            nxt.append(s)
                            yield
                        streams = nxt

                fill_it = merged(fills)
                atts = att_stream(j)
                n_att = 4 * (4 * (j + 1) + 1)       # steps + finalizes
                n_fill = (13 if j + 1 < nch else 0) + (8 if j >= 1 else 0)
                # first fill unit = xt DMA trigger: emit before attention
                done_f = 0
                try:
                    next(fill_it)
                    done_f += 1
                except StopIteration:
                    pass
                acc = 0.0
                rate = max(n_fill - done_f, 0) / max(n_att, 1)
                for _ in atts:
                    acc += rate
                    while acc >= 1.0:
                        acc -= 1.0
                        try:
                            next(fill_it)
                        except StopIteration:
                            break
                drain(fill_it)
            drain(proj_stream(nch - 1))

    if split_waits:  # breaks CoreSim's sem bookkeeping; needed for walrus
        _split_multiwaits(nc)
    return nc


def make_in_maps(x, W_qkv, b_qkv, W_pr):
    """Shard FULL inputs into the 8 per-core input dicts."""
    x = np.asarray(x, dtype=np.float32)
    W_qkv = np.asarray(W_qkv, dtype=np.float32)
    b_qkv = np.asarray(b_qkv, dtype=np.float32)
    W_pr = np.asarray(W_pr, dtype=np.float32)
    in_maps = []
    for core in range(8):
        b, g = divmod(core, 2)
        sl = slice(g * GC, (g + 1) * GC)
        in_maps.append(
            {
                "xT": np.ascontiguousarray(x[b].T).astype(NP_BF16),
                "wq": np.ascontiguousarray(W_qkv[:, 0 * C :][:, sl]).astype(NP_BF16),
                "wk": np.ascontiguousarray(W_qkv[:, 1 * C :][:, sl]).astype(NP_BF16),
                "wv": np.ascontiguousarray(W_qkv[:, 2 * C :][:, sl]).astype(NP_BF16),
                "bq": np.ascontiguousarray(b_qkv[0 * C :][sl]),
                "bk": np.ascontiguousarray(b_qkv[1 * C :][sl]),
                "bv": np.ascontiguousarray(b_qkv[2 * C :][sl]).astype(NP_BF16),
                "wpr": np.ascontiguousarray(W_pr[sl, :]).astype(NP_BF16),
            }
        )
    return in_maps


def assemble_output(parts, b_pr):
    """parts: 8 per-core outT [C, T] partials -> full [B, T, C] output."""
    b_pr = np.asarray(b_pr, dtype=np.float32)
    out = np.empty((B, T_FULL, C), dtype=np.float32)
    for b in range(B):
        out[b] = (parts[2 * b] + parts[2 * b + 1]).T + b_pr
    return out


_CACHE = {}


def kernel(x, W_qkv, b_qkv, W_pr, b_pr):
    if "nc" not in _CACHE:
        _CACHE["nc"] = build_attention(T_FULL)
    in_maps = make_in_maps(x, W_qkv, b_qkv, W_pr)
    res = run_bass_kernel_spmd(_CACHE["nc"], in_maps, core_ids=list(range(8)))
    parts = [r["outT"] for r in res.results]
    return assemble_output(parts, b_pr)


# revision 9
# speedup vs baseline: 1.1952x; 1.0458x over previous
"""Causal self-attention (B=4, T=2048, C=1024, H=16) on 8 TRN2 NeuronCores.

Sharding: core = (batch b, head-group g) with b = core//2, g = core%2.
Each core computes, for its batch and its 8 heads:
  QKV projection (W_qkv column shard), causal attention, and a PARTIAL
  output projection (W_pr row shard).  Host sums the two partials per
  batch and adds b_pr.

On-chip layout (per core):
  xT   [C, T]   : x[b].T               (DMA'd per 512-wide t-chunk)
  Q^T/K^T [512, T] : computed transposed (lhsT = W slice, rhs = xT)
  V    [T, 512] : computed natural     (lhsT = xT slice,  rhs = Wv)
  Attention per chunk j (q in [512j, 512j+512)):
    S^T blocks [k-tile 128, q 512] = K_h^T matmul pair packed in PE row
    groups (contraction d=64 at partition bases 0/64), exp on ACT
    (scale=1/8) -> A^T bf16, causal mask via gpsimd.affine_select over
    only the columns that can be masked.
    Y^T[65, q] += [V_h | ones].T @ A^T   (row 64 = softmax denominators)
  Per head pair: denominators to partitions {0, 64} of one staging
  tile, ONE Ln + ONE Exp(-x) (ACT cost is per-column, so batching
  partitions is free), 1/D broadcast via a packed pair of rank-1
  matmuls, then two PSUM*PSUM -> bf16 Y^T writes on DVE.
  Projection: out^T[c_out, t] += W_pr_shard.T @ Y^T

Software pipeline: the QKV sweep for chunk j+1 and the output
projection for chunk j-1 are interleaved into the attention
instruction stream of chunk j, so the PE's exp/denominator wait gaps
are filled with independent matmul work.
"""

import numpy as np

import concourse.bass as bass
import concourse.mybir as mybir
import concourse.tile as tile
from concourse.bass_utils import run_bass_kernel_spmd


def _split_multiwaits(nc: bass.Bass, max_waits: int = 1) -> None:
    """The walrus build in this container rejects >max_waits sync-waits on an
    instruction ("Too many sync wait commands").  Move extra waits onto
    same-engine NoOps inserted immediately before the instruction — the
    engine blocks on each NoOp's wait first, so semantics are unchanged."""
    n = 0
    for fn in nc.m.functions:
        for blk in fn.blocks:
            out = []
            for inst in blk.instructions:
                si = getattr(inst, "sync_info", None)
                waits = list(si.on_wait) if si is not None and si.on_wait else []
                if len(waits) > max_waits:
                    keep = waits[-max_waits:]
                    for w in waits[: -max_waits]:
                        nop = mybir.InstNoOp(name=f"{inst.name}-w{n}", ins=[], outs=[])
                        n += 1
                        nop.engine = inst.engine
                        nop.sync_info = mybir.SyncInfo(on_wait=[w], on_update=[])
                        out.append(nop)
                    inst.sync_info = mybir.SyncInfo(
                        on_wait=keep, on_update=list(si.on_update or [])
                    )
                out.append(inst)
            blk.instructions = out

AF = mybir.ActivationFunctionType
ALU = mybir.AluOpType

F32 = mybir.dt.float32
F32R = mybir.dt.float32r
BF16 = mybir.dt.bfloat16

B, T_FULL, C = 4, 2048, 1024
H, HD = 16, 64
HPC = 8              # heads per core
GC = HPC * HD        # 512: per-core head-group width
P = 128
CH = 512             # q-chunk width
NKC = C // P         # 8 k-tiles over the C contraction

NP_BF16 = mybir.dt.np(BF16)


def build_attention(T: int = T_FULL, split_waits: bool = True) -> bass.Bass:
    assert T % CH == 0
    nch = T // CH        # q-chunks
    ntt = T // P         # t-tiles

    nc = bass.Bass("TRN2", debug=False, num_devices=8)

    xT_d = nc.dram_tensor("xT", [C, T], BF16, kind="ExternalInput").ap()
    wq_d = nc.dram_tensor("wq", [C, GC], BF16, kind="ExternalInput").ap()
    wk_d = nc.dram_tensor("wk", [C, GC], BF16, kind="ExternalInput").ap()
    wv_d = nc.dram_tensor("wv", [C, GC], BF16, kind="ExternalInput").ap()
    bq_d = nc.dram_tensor("bq", [GC], F32, kind="ExternalInput").ap()
    bk_d = nc.dram_tensor("bk", [GC], F32, kind="ExternalInput").ap()
    bv_d = nc.dram_tensor("bv", [GC], BF16, kind="ExternalInput").ap()
    wpr_d = nc.dram_tensor("wpr", [GC, C], BF16, kind="ExternalInput").ap()
    out_d = nc.dram_tensor("outT", [C, T], F32, kind="ExternalOutput").ap()

    with tile.TileContext(nc) as tc:
        with (
            tc.tile_pool(name="singles", bufs=1) as singles,
            tc.tile_pool(name="xt", bufs=2) as xt_pool,
            tc.tile_pool(name="qt", bufs=2) as qt_pool,
            tc.tile_pool(name="at", bufs=5) as at_pool,
            tc.tile_pool(name="yt", bufs=2) as yt_pool,
            tc.tile_pool(name="dd", bufs=3) as d_pool,
            tc.tile_pool(name="ot", bufs=3) as out_pool,
            tc.tile_pool(name="swps", bufs=2, space="PSUM") as sweep_ps,
            tc.tile_pool(name="scps", bufs=2, space="PSUM") as sc_ps,
            tc.tile_pool(name="yps", bufs=2, space="PSUM") as y_ps,
        ):
            # ---- resident tensors; DMA issue order = priority order ----
            wq_sb = singles.tile([P, NKC, GC], BF16)
            nc.sync.dma_start(out=wq_sb, in_=wq_d.rearrange("(kc p) n -> p kc n", p=P))
            bqk_sb = singles.tile([P, 2, GC // P], F32)
            nc.sync.dma_start(
                out=bqk_sb[:, 0, :], in_=bq_d.rearrange("(m p) -> p m", p=P)
            )
            nc.sync.dma_start(
                out=bqk_sb[:, 1, :], in_=bk_d.rearrange("(m p) -> p m", p=P)
            )
            wk_sb = singles.tile([P, NKC, GC], BF16)
            wv_sb = singles.tile([P, NKC, GC], BF16)

            bv_sb = singles.tile([1, GC], BF16)
            wpr_sb = singles.tile([P, GC // P, C], BF16)

            ones_sb = singles.tile([1, P], BF16)
            nc.vector.memset(ones_sb, 1.0)
            ones64_sb = singles.tile([P, HD], BF16)
            nc.vector.memset(ones64_sb, 1.0)

            kt_sb = singles.tile([P, GC // P, T], BF16)       # K^T, filled per chunk
            v_sb = singles.tile([P, ntt, HPC, HD + 1], BF16)  # [V | 1] per t-tile/head
            nc.vector.memset(v_sb[:, :, :, HD : HD + 1], 1.0)

            chunk_qt = {}   # j -> qt tile
            chunk_yt = {}   # j -> yt tile

            def sweep_stream(j, first=False):
                """QT/KT chunk-j columns, V t-tiles 4j..4j+3.  Yields at
                unit boundaries (one PSUM pass-group per unit)."""
                tsl = slice(j * CH, (j + 1) * CH)
                xt_t = xt_pool.tile([P, NKC, CH], BF16)
                nc.sync.dma_start(
                    out=xt_t, in_=xT_d[:, tsl].rearrange("(kc p) t -> p kc t", p=P)
                )
                if first:
                    # remaining residents, behind xt0 on the sync queue
                    nc.sync.dma_start(
                        out=wk_sb, in_=wk_d.rearrange("(kc p) n -> p kc n", p=P)
                    )
                    nc.sync.dma_start(
                        out=wv_sb, in_=wv_d.rearrange("(kc p) n -> p kc n", p=P)
                    )
                    nc.sync.dma_start(
                        out=bv_sb, in_=bv_d.rearrange("(o n) -> o n", o=1)
                    )
                    nc.sync.dma_start(
                        out=wpr_sb, in_=wpr_d.rearrange("(kp p) m -> p kp m", p=P)
                    )
                yield

                qt_t = qt_pool.tile([P, GC // P, CH], BF16)
                chunk_qt[j] = qt_t
                for mq in range(GC // P):
                    ps = sweep_ps.tile([P, CH], F32, tag="ps")
                    for kc in range(NKC):
                        nc.tensor.matmul(
                            ps,
                            lhsT=wq_sb[:, kc, mq * P : (mq + 1) * P],
                            rhs=xt_t[:, kc, :],
                            start=(kc == 0),
                            stop=(kc == NKC - 1),
                        )
                    nc.vector.tensor_scalar_add(
                        out=qt_t[:, mq, :], in0=ps, scalar1=bqk_sb[:, 0, mq : mq + 1]
                    )
                    yield
                for mk in range(GC // P):
                    ps = sweep_ps.tile([P, CH], F32, tag="ps")
                    for kc in range(NKC):
                        nc.tensor.matmul(
                            ps,
                            lhsT=wk_sb[:, kc, mk * P : (mk + 1) * P],
                            rhs=xt_t[:, kc, :],
                            start=(kc == 0),
                            stop=(kc == NKC - 1),
                        )
                    nc.vector.tensor_scalar_add(
                        out=kt_sb[:, mk, tsl], in0=ps, scalar1=bqk_sb[:, 1, mk : mk + 1]
                    )
                    yield
                for tl in range(4):
                    tt = 4 * j + tl
                    ps = sweep_ps.tile([P, GC], F32, tag="ps")
                    for kc in range(NKC):
                        nc.tensor.matmul(
                            ps,
                            lhsT=xt_t[:, kc, tl * P : (tl + 1) * P],
                            rhs=wv_sb[:, kc, :],
                            start=(kc == 0),
                            stop=False,
                        )
                    # bias row: V += ones.T @ bv
                    nc.tensor.matmul(
                        ps, lhsT=ones_sb, rhs=bv_sb, start=False, stop=True
                    )
                    nc.vector.tensor_copy(
                        v_sb[:, tt, :, 0:HD], ps.rearrange("p (h d) -> p h d", h=HPC)
                    )
                    yield

            def att_stream(j):
                """Attention for chunk j.  Head pairs at PE row groups 0/64."""
                nkt = 4 * (j + 1)
                yt_t = yt_pool.tile([P, GC // P, CH], BF16)
                chunk_yt[j] = yt_t
                qt_t = chunk_qt[j]
                for hp in range(HPC // 2):
                    mk = hp
                    yps0 = y_ps.tile([P, CH], F32, tag="yps")  # rows 0..64
                    yps1 = y_ps.tile([P, CH], F32, tag="yps")
                    ypss = [yps0, yps1]
                    for ki in range(nkt):
                        # diagonal k-tile r: columns q < 128r are entirely
                        # masked, so scores/exp run on q >= 128r only and
                        # the mask is always exactly 128 columns wide.
                        r = ki - 4 * j
                        q0 = P * r if r > 0 else 0
                        sc = sc_ps.tile([P, 2 * CH], F32)
                        for u in range(2):  # u = head parity; po = 64*u
                            po = HD * u
                            nc.tensor.matmul(
                                sc[:, u * CH + q0 : (u + 1) * CH],
                                lhsT=kt_sb[po : po + HD, mk, ki * P : (ki + 1) * P],
                                rhs=qt_t[po : po + HD, mk, q0:],
                                start=True,
                                stop=True,
                            )
                        at_t = at_pool.tile([P, 2 * CH], BF16)
                        for u in range(2):
                            nc.scalar.activation(
                                at_t[:, u * CH + q0 : (u + 1) * CH],
                                sc[:, u * CH + q0 : (u + 1) * CH],
                                AF.Exp,
                                scale=0.125,
                            )
                            if q0 > 0:  # zero the skipped fully-masked cols
                                nc.vector.memset(
                                    at_t[:, u * CH : u * CH + q0], 0.0
                                )
                        if r >= 0:  # partially-masked 128 cols: zero q < k
                            for u in range(2):
                                blk = at_t[:, u * CH + q0 : u * CH + q0 + P]
                                nc.gpsimd.affine_select(
                                    out=blk,
                                    in_=blk,
                                    pattern=[[1, P]],
                                    compare_op=ALU.is_ge,
                                    fill=0.0,
                                    base=0,
                                    channel_multiplier=-1,
                                )
                        for u in range(2):
                            nc.tensor.matmul(
                                ypss[u][0 : HD + 1, :],
                                lhsT=v_sb[:, ki, 2 * hp + u, :],
                                rhs=at_t[:, u * CH : (u + 1) * CH],
                                start=(ki == 0),
                                stop=(ki == nkt - 1),
                            )
                        yield
                    # ---- head-pair softmax denominators, batched ----
                    # D rows to partitions {0, 64} of one staging tile; ACT
                    # ops cost per-column, so one Ln + one Exp covers both.
                    dsb = d_pool.tile([P, CH], F32, tag="dsb")
                    nc.vector.tensor_copy(dsb[0:1, :], yps0[HD : HD + 1, :])
                    nc.vector.tensor_copy(dsb[64:65, :], yps1[HD : HD + 1, :])
                    dln = d_pool.tile([P, CH], F32, tag="dln")
                    nc.scalar.activation(dln[0:65, :], dsb[0:65, :], AF.Ln)
                    dinv = d_pool.tile([P, CH], BF16, tag="dinv")
                    nc.scalar.activation(
                        dinv[0:65, :], dln[0:65, :], AF.Exp, scale=-1.0
                    )
                    # let the scheduler slot PE fill work behind the ACT chain
                    yield 2
                    # 1/D broadcast: packed pair of rank-1 matmuls
                    bc_ps = sweep_ps.tile([P, CH], F32, tag="ps")
                    nc.tensor.matmul(
                        bc_ps[0:HD, :],
                        lhsT=ones64_sb[0:1, :],
                        rhs=dinv[0:1, :],
                        start=True,
                        stop=True,
                    )
                    nc.tensor.matmul(
                        bc_ps[HD : 2 * HD, :],
                        lhsT=ones64_sb[64:65, :],
                        rhs=dinv[64:65, :],
                        start=True,
                        stop=True,
                    )
                    # walrus rejects tensor_tensor with two PSUM sources, so
                    # stage the broadcast through SBUF (one copy per pair)
                    bc_t = d_pool.tile([P, CH], F32, tag="bc")
                    nc.vector.tensor_copy(bc_t[0 : 2 * HD, :], bc_ps[0 : 2 * HD, :])
                    nc.vector.tensor_mul(
                        yt_t[0:HD, mk, :], yps0[0:HD, :], bc_t[0:HD, :]
                    )
                    nc.vector.tensor_mul(
                        yt_t[HD:P, mk, :], yps1[0:HD, :], bc_t[HD : 2 * HD, :]
                    )
                    yield

            def proj_stream(j):
                """Partial output projection for chunk j (bf16)."""
                tsl = slice(j * CH, (j + 1) * CH)
                yt_t = chunk_yt[j]
                for m in range(C // P):
                    pp = sweep_ps.tile([P, CH], F32, tag="ps")
                    for kp in range(GC // P):
                        nc.tensor.matmul(
                            pp,
                            lhsT=wpr_sb[:, kp, m * P : (m + 1) * P],
                            rhs=yt_t[:, kp, :],
                            start=(kp == 0),
                            stop=(kp == GC // P - 1),
                        )
                        if kp == 1:
                            yield
                    ot = out_pool.tile([P, CH], F32)
                    nc.vector.tensor_copy(ot, pp)
                    nc.sync.dma_start(out=out_d[m * P : (m + 1) * P, tsl], in_=ot)
                    yield

            def drain(it):
                for _ in it:
                    pass

            # ---- schedule ----
            drain(sweep_stream(0, first=True))
            for j in range(nch):
                fills = []
                if j + 1 < nch:
                    fills.append(sweep_stream(j + 1))
                if j >= 1:
                    fills.append(proj_stream(j - 1))
                # merged fill iterator, round-robin across fill streams
                def merged(streams):
                    while streams:
                        nxt = []
                        for s in streams:
                            try:
                                next(s)
                            except StopIteration:
                                continue
                            nxt.append(s)
                            yield
                        streams = nxt

                fill_it = merged(fills)
                n_steps = 16 * (j + 1)              # (hp, ki) attention steps
                n_fill = (13 if j + 1 < nch else 0) + (24 if j >= 1 else 0)
                # first fill unit = xt DMA trigger: emit before attention;
                # reserve 2 fills per head-pair finalize (tagged yields)
                done_f = 0
                try:
                    next(fill_it)
                    done_f += 1
                except StopIteration:
                    pass
                acc = 0.0
                rate = max(n_fill - done_f - 2 * 4, 0) / max(n_steps, 1)
                for want in att_stream(j):
                    take = want if want else 0
                    if not want:
                        acc += rate
                        while acc >= 1.0:
                            acc -= 1.0
                            take += 1
                    for _ in range(take):
                        try:
                            next(fill_it)
                        except StopIteration:
                            break
                drain(fill_it)
            drain(proj_stream(nch - 1))

    if split_waits:  # breaks CoreSim's sem bookkeeping; needed for walrus
        _split_multiwaits(nc)
    return nc


def make_in_maps(x, W_qkv, b_qkv, W_pr):
    """Shard FULL inputs into the 8 per-core input dicts."""
    x = np.asarray(x, dtype=np.float32)
    W_qkv = np.asarray(W_qkv, dtype=np.float32)
    b_qkv = np.asarray(b_qkv, dtype=np.float32)
    W_pr = np.asarray(W_pr, dtype=np.float32)
    in_maps = []
    for core in range(8):
        b, g = divmod(core, 2)
        sl = slice(g * GC, (g + 1) * GC)
        in_maps.append(
            {
                "xT": np.ascontiguousarray(x[b].T).astype(NP_BF16),
                "wq": np.ascontiguousarray(W_qkv[:, 0 * C :][:, sl]).astype(NP_BF16),
                "wk": np.ascontiguousarray(W_qkv[:, 1 * C :][:, sl]).astype(NP_BF16),
                "wv": np.ascontiguousarray(W_qkv[:, 2 * C :][:, sl]).astype(NP_BF16),
                "bq": np.ascontiguousarray(b_qkv[0 * C :][sl]),
                "bk": np.ascontiguousarray(b_qkv[1 * C :][sl]),
                "bv": np.ascontiguousarray(b_qkv[2 * C :][sl]).astype(NP_BF16),
                "wpr": np.ascontiguousarray(W_pr[sl, :]).astype(NP_BF16),
            }
        )
    return in_maps


def assemble_output(parts, b_pr):
    """parts: 8 per-core outT [C, T] partials -> full [B, T, C] output."""
    b_pr = np.asarray(b_pr, dtype=np.float32)
    out = np.empty((B, T_FULL, C), dtype=np.float32)
    for b in range(B):
        out[b] = (parts[2 * b] + parts[2 * b + 1]).T + b_pr
    return out


_CACHE = {}


def kernel(x, W_qkv, b_qkv, W_pr, b_pr):
    if "nc" not in _CACHE:
        _CACHE["nc"] = build_attention(T_FULL)
    in_maps = make_in_maps(x, W_qkv, b_qkv, W_pr)
    res = run_bass_kernel_spmd(_CACHE["nc"], in_maps, core_ids=list(range(8)))
    parts = [r["outT"] for r in res.results]
    return assemble_output(parts, b_pr)


# revision 13
# speedup vs baseline: 1.2882x; 1.0778x over previous
"""Causal self-attention (B=4, T=2048, C=1024, H=16) on 8 TRN2 NeuronCores.

Sharding: core = (batch b, head-group g) with b = core//2, g = core%2.
Each core computes, for its batch and its 8 heads:
  QKV projection (W_qkv column shard), causal attention, and a PARTIAL
  output projection (W_pr row shard).  Host sums the two partials per
  batch and adds b_pr.

On-chip layout (per core):
  xT   [C, T]   : x[b].T               (DMA'd per 512-wide t-chunk)
  Q^T/K^T [512, T] : computed transposed (lhsT = W slice, rhs = xT)
  V    [T, 512] : computed natural     (lhsT = xT slice,  rhs = Wv)
  Attention per chunk j (q in [512j, 512j+512)):
    S^T blocks [k-tile 128, q 512] = K_h^T matmul pair packed in PE row
    groups (contraction d=64 at partition bases 0/64), exp on ACT
    (scale=1/8) -> A^T bf16, causal mask via gpsimd.affine_select over
    only the columns that can be masked.
    Y^T[65, q] += [V_h | ones].T @ A^T   (row 64 = softmax denominators)
  Per head pair: denominators to partitions {0, 64} of one staging
  tile, ONE Ln + ONE Exp(-x) (ACT cost is per-column, so batching
  partitions is free), 1/D broadcast via a packed pair of rank-1
  matmuls, then two PSUM*PSUM -> bf16 Y^T writes on DVE.
  Projection: out^T[c_out, t] += W_pr_shard.T @ Y^T

Software pipeline: the QKV sweep for chunk j+1 and the output
projection for chunk j-1 are interleaved into the attention
instruction stream of chunk j, so the PE's exp/denominator wait gaps
are filled with independent matmul work.
"""

import numpy as np

import concourse.bass as bass
import concourse.mybir as mybir
import concourse.tile as tile
from concourse.bass_utils import run_bass_kernel_spmd


def _split_multiwaits(nc: bass.Bass, max_waits: int = 1) -> None:
    """The walrus build in this container rejects >max_waits sync-waits on an
    instruction ("Too many sync wait commands").  Move extra waits onto
    same-engine NoOps inserted immediately before the instruction — the
    engine blocks on each NoOp's wait first, so semantics are unchanged."""
    n = 0
    for fn in nc.m.functions:
        for blk in fn.blocks:
            out = []
            for inst in blk.instructions:
                si = getattr(inst, "sync_info", None)
                waits = list(si.on_wait) if si is not None and si.on_wait else []
                if len(waits) > max_waits:
                    keep = waits[-max_waits:]
                    for w in waits[: -max_waits]:
                        nop = mybir.InstNoOp(name=f"{inst.name}-w{n}", ins=[], outs=[])
                        n += 1
                        nop.engine = inst.engine
                        nop.sync_info = mybir.SyncInfo(on_wait=[w], on_update=[])
                        out.append(nop)
                    inst.sync_info = mybir.SyncInfo(
                        on_wait=keep, on_update=list(si.on_update or [])
                    )
                out.append(inst)
            blk.instructions = out

AF = mybir.ActivationFunctionType
ALU = mybir.AluOpType

F32 = mybir.dt.float32
F32R = mybir.dt.float32r
BF16 = mybir.dt.bfloat16

B, T_FULL, C = 4, 2048, 1024
H, HD = 16, 64
HPC = 8              # heads per core
GC = HPC * HD        # 512: per-core head-group width
P = 128
CH = 512             # q-chunk width
NKC = C // P         # 8 k-tiles over the C contraction

NP_BF16 = mybir.dt.np(BF16)


def build_attention(T: int = T_FULL, split_waits: bool = True) -> bass.Bass:
    assert T % CH == 0
    nch = T // CH        # q-chunks
    ntt = T // P         # t-tiles

    nc = bass.Bass("TRN2", debug=False, num_devices=8)

    xT_d = nc.dram_tensor("xT", [C, T], BF16, kind="ExternalInput").ap()
    wq_d = nc.dram_tensor("wq", [C, GC], BF16, kind="ExternalInput").ap()
    wk_d = nc.dram_tensor("wk", [C, GC], BF16, kind="ExternalInput").ap()
    wv_d = nc.dram_tensor("wv", [C, GC], BF16, kind="ExternalInput").ap()
    bq_d = nc.dram_tensor("bq", [GC], F32, kind="ExternalInput").ap()
    bk_d = nc.dram_tensor("bk", [GC], F32, kind="ExternalInput").ap()
    bv_d = nc.dram_tensor("bv", [GC], BF16, kind="ExternalInput").ap()
    wpr_d = nc.dram_tensor("wpr", [GC, C], BF16, kind="ExternalInput").ap()
    out_d = nc.dram_tensor("outT", [C, T], F32, kind="ExternalOutput").ap()

    with tile.TileContext(nc) as tc:
        with (
            tc.tile_pool(name="singles", bufs=1) as singles,
            tc.tile_pool(name="xt", bufs=2) as xt_pool,
            tc.tile_pool(name="qt", bufs=2) as qt_pool,
            tc.tile_pool(name="at", bufs=5) as at_pool,
            tc.tile_pool(name="yt", bufs=2) as yt_pool,
            tc.tile_pool(name="dd", bufs=3) as d_pool,
            tc.tile_pool(name="ot", bufs=3) as out_pool,
            tc.tile_pool(name="swps", bufs=2, space="PSUM") as sweep_ps,
            tc.tile_pool(name="scps", bufs=2, space="PSUM") as sc_ps,
            tc.tile_pool(name="yps", bufs=2, space="PSUM") as y_ps,
        ):
            # ---- resident tensors; DMA issue order = priority order ----
            wq_sb = singles.tile([P, NKC, GC], BF16)
            nc.sync.dma_start(out=wq_sb, in_=wq_d.rearrange("(kc p) n -> p kc n", p=P))
            bqk_sb = singles.tile([P, 2, GC // P], F32)
            nc.sync.dma_start(
                out=bqk_sb[:, 0, :], in_=bq_d.rearrange("(m p) -> p m", p=P)
            )
            nc.sync.dma_start(
                out=bqk_sb[:, 1, :], in_=bk_d.rearrange("(m p) -> p m", p=P)
            )
            wk_sb = singles.tile([P, NKC, GC], BF16)
            wv_sb = singles.tile([P, NKC, GC], BF16)

            bv_sb = singles.tile([1, GC], BF16)
            wpr_sb = singles.tile([P, GC // P, C], BF16)

            ones_sb = singles.tile([1, P], BF16)
            nc.vector.memset(ones_sb, 1.0)
            ones64_sb = singles.tile([P, HD], BF16)
            nc.vector.memset(ones64_sb, 1.0)

            kt_sb = singles.tile([P, GC // P, T], BF16)       # K^T, filled per chunk
            v_sb = singles.tile([P, ntt, HPC, HD + 1], BF16)  # [V | 1] per t-tile/head
            nc.vector.memset(v_sb[:, :, :, HD : HD + 1], 1.0)

            chunk_qt = {}   # j -> qt tile
            chunk_yt = {}   # j -> yt tile

            def sweep_stream(j, first=False):
                """QT/KT chunk-j columns, V t-tiles 4j..4j+3.  Yields at
                unit boundaries (one PSUM pass-group per unit)."""
                tsl = slice(j * CH, (j + 1) * CH)
                xt_t = xt_pool.tile([P, NKC, CH], BF16)
                nc.sync.dma_start(
                    out=xt_t, in_=xT_d[:, tsl].rearrange("(kc p) t -> p kc t", p=P)
                )
                if first:
                    # remaining residents, behind xt0 on the sync queue
                    nc.sync.dma_start(
                        out=wk_sb, in_=wk_d.rearrange("(kc p) n -> p kc n", p=P)
                    )
                    nc.sync.dma_start(
                        out=wv_sb, in_=wv_d.rearrange("(kc p) n -> p kc n", p=P)
                    )
                    nc.sync.dma_start(
                        out=bv_sb, in_=bv_d.rearrange("(o n) -> o n", o=1)
                    )
                    nc.sync.dma_start(
                        out=wpr_sb, in_=wpr_d.rearrange("(kp p) m -> p kp m", p=P)
                    )
                yield

                qt_t = qt_pool.tile([P, GC // P, CH], BF16)
                chunk_qt[j] = qt_t
                for mq in range(GC // P):
                    ps = sweep_ps.tile([P, CH], F32, tag="ps")
                    for kc in range(NKC):
                        nc.tensor.matmul(
                            ps,
                            lhsT=wq_sb[:, kc, mq * P : (mq + 1) * P],
                            rhs=xt_t[:, kc, :],
                            start=(kc == 0),
                            stop=(kc == NKC - 1),
                        )
                    nc.vector.tensor_scalar_add(
                        out=qt_t[:, mq, :], in0=ps, scalar1=bqk_sb[:, 0, mq : mq + 1]
                    )
                    yield
                for mk in range(GC // P):
                    ps = sweep_ps.tile([P, CH], F32, tag="ps")
                    for kc in range(NKC):
                        nc.tensor.matmul(
                            ps,
                            lhsT=wk_sb[:, kc, mk * P : (mk + 1) * P],
                            rhs=xt_t[:, kc, :],
                            start=(kc == 0),
                            stop=(kc == NKC - 1),
                        )
                    nc.vector.tensor_scalar_add(
                        out=kt_sb[:, mk, tsl], in0=ps, scalar1=bqk_sb[:, 1, mk : mk + 1]
                    )
                    yield
                for tl in range(4):
                    tt = 4 * j + tl
                    ps = sweep_ps.tile([P, GC], F32, tag="ps")
                    for kc in range(NKC):
                        nc.tensor.matmul(
                            ps,
                            lhsT=xt_t[:, kc, tl * P : (tl + 1) * P],
                            rhs=wv_sb[:, kc, :],
                            start=(kc == 0),
                            stop=False,
                        )
                    # bias row: V += ones.T @ bv
                    nc.tensor.matmul(
                        ps, lhsT=ones_sb, rhs=bv_sb, start=False, stop=True
                    )
                    nc.vector.tensor_copy(
                        v_sb[:, tt, :, 0:HD], ps.rearrange("p (h d) -> p h d", h=HPC)
                    )
                    yield

            def att_stream(j):
                """Attention for chunk j.  Head pairs at PE row groups 0/64."""
                nkt = 4 * (j + 1)
                yt_t = yt_pool.tile([P, GC // P, CH], BF16)
                chunk_yt[j] = yt_t
                qt_t = chunk_qt[j]
                for hp in range(HPC // 2):
                    mk = hp
                    yps0 = y_ps.tile([P, CH], F32, tag="yps")  # rows 0..64
                    yps1 = y_ps.tile([P, CH], F32, tag="yps")
                    ypss = [yps0, yps1]
                    for ki in range(nkt):
                        # diagonal k-tile r: columns q < 128r are entirely
                        # masked, so scores/exp run on q >= 128r only and
                        # the mask is always exactly 128 columns wide.
                        r = ki - 4 * j
                        q0 = P * r if r > 0 else 0
                        sc = sc_ps.tile([P, 2 * CH], F32)
                        for u in range(2):  # u = head parity; po = 64*u
                            po = HD * u
                            nc.tensor.matmul(
                                sc[:, u * CH + q0 : (u + 1) * CH],
                                lhsT=kt_sb[po : po + HD, mk, ki * P : (ki + 1) * P],
                                rhs=qt_t[po : po + HD, mk, q0:],
                                start=True,
                                stop=True,
                            )
                        at_t = at_pool.tile([P, 2 * CH], BF16)
                        if q0 == 0:
                            nc.scalar.activation(at_t, sc, AF.Exp, scale=0.125)
                        else:
                            for u in range(2):
                                nc.scalar.activation(
                                    at_t[:, u * CH + q0 : (u + 1) * CH],
                                    sc[:, u * CH + q0 : (u + 1) * CH],
                                    AF.Exp,
                                    scale=0.125,
                                )
                                # zero the skipped fully-masked cols
                                nc.gpsimd.memset(
                                    at_t[:, u * CH : u * CH + q0], 0.0
                                )
                        if r >= 0:  # partially-masked 128 cols: zero q < k
                            for u in range(2):
                                blk = at_t[:, u * CH + q0 : u * CH + q0 + P]
                                nc.gpsimd.affine_select(
                                    out=blk,
                                    in_=blk,
                                    pattern=[[1, P]],
                                    compare_op=ALU.is_ge,
                                    fill=0.0,
                                    base=0,
                                    channel_multiplier=-1,
                                )
                        for u in range(2):
                            nc.tensor.matmul(
                                ypss[u][0 : HD + 1, :],
                                lhsT=v_sb[:, ki, 2 * hp + u, :],
                                rhs=at_t[:, u * CH : (u + 1) * CH],
                                start=(ki == 0),
                                stop=(ki == nkt - 1),
                            )
                        yield
                    # ---- head-pair softmax denominators, batched ----
                    # D rows to partitions {0, 64} of one staging tile; ACT
                    # ops cost per-column, so one Ln + one Exp covers both.
                    dsb = d_pool.tile([P, CH], F32, tag="dsb")
                    nc.vector.tensor_copy(dsb[0:1, :], yps0[HD : HD + 1, :])
                    nc.vector.tensor_copy(dsb[64:65, :], yps1[HD : HD + 1, :])
                    dln = d_pool.tile([P, CH], F32, tag="dln")
                    nc.scalar.activation(dln[0:65, :], dsb[0:65, :], AF.Ln)
                    dinv = d_pool.tile([P, CH], BF16, tag="dinv")
                    nc.scalar.activation(
                        dinv[0:65, :], dln[0:65, :], AF.Exp, scale=-1.0
                    )
                    # let the scheduler slot PE fill work behind the ACT chain
                    yield 2
                    # 1/D broadcast: packed pair of rank-1 matmuls
                    bc_ps = sweep_ps.tile([P, CH], F32, tag="ps")
                    nc.tensor.matmul(
                        bc_ps[0:HD, :],
                        lhsT=ones64_sb[0:1, :],
                        rhs=dinv[0:1, :],
                        start=True,
                        stop=True,
                    )
                    nc.tensor.matmul(
                        bc_ps[HD : 2 * HD, :],
                        lhsT=ones64_sb[64:65, :],
                        rhs=dinv[64:65, :],
                        start=True,
                        stop=True,
                    )
                    # walrus rejects tensor_tensor with two PSUM sources, so
                    # stage the broadcast through SBUF (one copy per pair)
                    bc_t = d_pool.tile([P, CH], F32, tag="bc")
                    nc.vector.tensor_copy(bc_t[0 : 2 * HD, :], bc_ps[0 : 2 * HD, :])
                    nc.vector.tensor_mul(
                        yt_t[0:HD, mk, :], yps0[0:HD, :], bc_t[0:HD, :]
                    )
                    nc.vector.tensor_mul(
                        yt_t[HD:P, mk, :], yps1[0:HD, :], bc_t[HD : 2 * HD, :]
                    )
                    yield

            def proj_stream(j):
                """Partial output projection for chunk j (bf16)."""
                tsl = slice(j * CH, (j + 1) * CH)
                yt_t = chunk_yt[j]
                for m in range(C // P):
                    pp = sweep_ps.tile([P, CH], F32, tag="ps")
                    for kp in range(GC // P):
                        nc.tensor.matmul(
                            pp,
                            lhsT=wpr_sb[:, kp, m * P : (m + 1) * P],
                            rhs=yt_t[:, kp, :],
                            start=(kp == 0),
                            stop=(kp == GC // P - 1),
                        )
                        if kp == 1:
                            yield
                    ot = out_pool.tile([P, CH], F32)
                    nc.vector.tensor_copy(ot, pp)
                    nc.sync.dma_start(out=out_d[m * P : (m + 1) * P, tsl], in_=ot)
                    yield

            def drain(it):
                for _ in it:
                    pass

            # ---- schedule ----
            drain(sweep_stream(0, first=True))
            for j in range(nch):
                fills = []
                if j + 1 < nch:
                    fills.append(sweep_stream(j + 1))
                if j >= 1:
                    fills.append(proj_stream(j - 1))
                # merged fill iterator, round-robin across fill streams
                def merged(streams):
                    while streams:
                        nxt = []
                        for s in streams:
                            try:
                                next(s)
                            except StopIteration:
                                continue
                            nxt.append(s)
                            yield
                        streams = nxt

                fill_it = merged(fills)
                n_steps = 16 * (j + 1)              # (hp, ki) attention steps
                n_fill = (13 if j + 1 < nch else 0) + (16 if j >= 1 else 0)
                # first fill unit = xt DMA trigger: emit before attention;
                # reserve 2 fills per head-pair finalize (tagged yields)
                done_f = 0
                try:
                    next(fill_it)
                    done_f += 1
                except StopIteration:
                    pass
                acc = 0.0
                rate = max(n_fill - done_f - 2 * 4, 0) / max(n_steps, 1)
                for want in att_stream(j):
                    take = want if want else 0
                    if not want:
                        acc += rate
                        while acc >= 1.0:
                            acc -= 1.0
                            take += 1
                    for _ in range(take):
                        try:
                            next(fill_it)
                        except StopIteration:
                            break
                drain(fill_it)
            drain(proj_stream(nch - 1))

    if split_waits:  # breaks CoreSim's sem bookkeeping; needed for walrus
        _split_multiwaits(nc)
    return nc


def make_in_maps(x, W_qkv, b_qkv, W_pr):
    """Shard FULL inputs into the 8 per-core input dicts."""
    x = np.asarray(x, dtype=np.float32)
    W_qkv = np.asarray(W_qkv, dtype=np.float32)
    b_qkv = np.asarray(b_qkv, dtype=np.float32)
    W_pr = np.asarray(W_pr, dtype=np.float32)
    in_maps = []
    for core in range(8):
        b, g = divmod(core, 2)
        sl = slice(g * GC, (g + 1) * GC)
        in_maps.append(
            {
                "xT": np.ascontiguousarray(x[b].T).astype(NP_BF16),
                "wq": np.ascontiguousarray(W_qkv[:, 0 * C :][:, sl]).astype(NP_BF16),
                "wk": np.ascontiguousarray(W_qkv[:, 1 * C :][:, sl]).astype(NP_BF16),
                "wv": np.ascontiguousarray(W_qkv[:, 2 * C :][:, sl]).astype(NP_BF16),
                "bq": np.ascontiguousarray(b_qkv[0 * C :][sl]),
                "bk": np.ascontiguousarray(b_qkv[1 * C :][sl]),
                "bv": np.ascontiguousarray(b_qkv[2 * C :][sl]).astype(NP_BF16),
                "wpr": np.ascontiguousarray(W_pr[sl, :]).astype(NP_BF16),
            }
        )
    return in_maps


def assemble_output(parts, b_pr):
    """parts: 8 per-core outT [C, T] partials -> full [B, T, C] output."""
    b_pr = np.asarray(b_pr, dtype=np.float32)
    out = np.empty((B, T_FULL, C), dtype=np.float32)
    for b in range(B):
        out[b] = (parts[2 * b] + parts[2 * b + 1]).T + b_pr
    return out


_CACHE = {}


def kernel(x, W_qkv, b_qkv, W_pr, b_pr):
    if "nc" not in _CACHE:
        _CACHE["nc"] = build_attention(T_FULL)
    in_maps = make_in_maps(x, W_qkv, b_qkv, W_pr)
    res = run_bass_kernel_spmd(_CACHE["nc"], in_maps, core_ids=list(range(8)))
    parts = [r["outT"] for r in res.results]
    return assemble_output(parts, b_pr)
